# revision 3
# baseline (speedup 1.0000x reference)
"""HGNN metapath GRU + edge-softmax message passing on 8 TRN2 NeuronCores.

Strategy (self-contained, full inputs in / full output out):
 - Edges sharded by DESTINATION NODE RANGE: core c owns nodes
   [c*2500, (c+1)*2500) and every edge whose dst lands there (host sorts
   edges by dst).  All segment ops are core-local: zero collectives.
 - Phase 1 builds the node embedding table from a host-pretransposed x
   (pure layout change), so no PE transposes are needed: two K=128
   matmuls per 128-node chunk write node-major emb rows directly.
 - GRU runs feature-major; the recurrent (W_hh) matmuls run in fp8-e4m3
   DoubleRow mode (2 k-chunks per instruction); h is kept in bf16 for
   the update equation and mirrored to fp8 for the matmul operand.
 - The n-gate add (xn + r*hn) is done on the PE via an identity-matmul
   accumulate into the xn psum, freeing vector-engine cycles.
 - Attention + one-hot scatter are fused into the per-tile loop; exp is
   computed as (1+tanh(a/2))/(1-tanh(a/2)) so the whole kernel uses one
   ACT table set (sigmoid/tanh) with no table reloads.
"""

import sys
import numpy as np

sys.path.insert(0, "/opt/trn_rl_repo")

import ml_dtypes  # noqa: E402

N_NODES = 20000
N_CORES = 8
NPC = N_NODES // N_CORES          # 2500 nodes per core
NODE_CHUNKS = (NPC + 127) // 128  # 20
WALK = 4
FEAT = 256
HID = 64
NH = 8
HR = NH * HID                     # 512
G3 = 3 * HR                       # 1536
OUT_DIM = 16
E_TILE = 512
NP_PAD = ((N_NODES + 511) // 512) * 512  # 20480 padded node rows

bf = ml_dtypes.bfloat16


def _wrap_idx(v):
    """int array [n] -> wrapped int16 [128, n//16] layout for dma_gather."""
    n = v.shape[0]
    assert n % 16 == 0
    w = v.reshape(n // 16, 16).T.astype(np.int16)      # [16, n//16]
    return np.tile(w, (8, 1))                           # [128, n//16]


def _host_prep(x, W_mlp, b_mlp, W_ih, W_hh, b_ih, b_hh, attn, W_emb, b_emb,
               W_last, b_last, edge_metapath_indices):
    idx = np.asarray(edge_metapath_indices).astype(np.int64)
    dst = idx[:, -1]
    core = np.clip(dst // NPC, 0, N_CORES - 1)

    per_core_eids = []
    for c in range(N_CORES):
        sel = np.nonzero(core == c)[0]
        order = np.argsort(dst[sel], kind="stable")
        per_core_eids.append(sel[order])
    counts = [len(e) for e in per_core_eids]
    E_pad = max(512, ((max(counts) + E_TILE - 1) // E_TILE) * E_TILE)
    n_tiles = E_pad // E_TILE
    n_ech = E_pad // 128

    sidx = np.zeros((N_CORES, E_pad, WALK), np.int64)
    ldst = np.full((N_CORES, E_pad), -1000, np.int64)
    for c in range(N_CORES):
        e = per_core_eids[c]
        sidx[c, :len(e)] = idx[e]
        ldst[c, :len(e)] = dst[e] - c * NPC

    gidx = np.zeros((N_CORES, n_tiles, 128, (WALK * E_TILE) // 16), np.int16)
    for c in range(N_CORES):
        for t in range(n_tiles):
            v = sidx[c, t * E_TILE:(t + 1) * E_TILE, :].T.reshape(-1)
            gidx[c, t] = _wrap_idx(v)

    # shared scatter schedule: union over cores of node-chunks touched per
    # edge-chunk (SPMD: one program, so the schedule must cover all cores)
    pairs = []
    pair_of = {}
    for k in range(n_ech):
        js = set()
        for c in range(N_CORES):
            d = ldst[c, k * 128:(k + 1) * 128]
            js |= set((d[d >= 0] // 128).tolist())
        if js:
            for j in range(min(js), max(js) + 1):
                pair_of[(k, j)] = len(pairs)
                pairs.append((k, j))
    last_k = {}
    for (k, j) in pairs:
        last_k[j] = k
    n_pairs = len(pairs)
    # flush node-chunk j after the tile containing its last edge-chunk
    flush_tile = {j: (last_k[j] // 4) for j in last_k}
    flush_at = {t: [] for t in range(n_tiles)}
    for j in range(NODE_CHUNKS):
        flush_at[flush_tile.get(j, 0)].append(j)

    oneh = np.zeros((N_CORES, max(n_pairs, 1), 128, 128), bf)
    m_ids = np.arange(128)
    for c in range(N_CORES):
        for p, (k, j) in enumerate(pairs):
            d = ldst[c, k * 128:(k + 1) * 128]
            oneh[c, p] = (d[:, None] == (j * 128 + m_ids)[None, :]).astype(bf)

    # weights
    Wc = (np.asarray(W_last, np.float32) @ np.asarray(W_emb, np.float32))
    BA = np.zeros((HR, 136), np.float32)
    attn = np.asarray(attn, np.float32)
    for h in range(NH):
        BA[h * HID:(h + 1) * HID, h * OUT_DIM:(h + 1) * OUT_DIM] = \
            Wc[:, h * HID:(h + 1) * HID].T
        BA[h * HID:(h + 1) * HID, 128 + h] = attn[0, h, :]
    ba_p = BA.reshape(4, 128, 136).transpose(1, 0, 2).reshape(128, 4 * 136).astype(bf)

    W_hhT = np.asarray(W_hh, np.float32).T                       # [512, 1536]
    whh_p = W_hhT.reshape(4, 128, G3).transpose(1, 0, 2).reshape(128, 4 * G3)
    whh8_p = whh_p.astype(ml_dtypes.float8_e4m3)
    wih_p = np.asarray(W_ih, np.float32).T.astype(bf)            # [64, 1536]

    b_ih = np.asarray(b_ih, np.float32)
    b_hh = np.asarray(b_hh, np.float32)
    brz = (b_ih + b_hh)[:2 * HR].reshape(8, 128).T.copy()        # [128, 8]
    bnih = b_ih[2 * HR:].reshape(4, 128).T.copy()                # [128, 4]
    bnhh = b_hh[2 * HR:].reshape(4, 128).T.copy()                # [128, 4]
    has_bnhh = bool(np.any(bnhh != 0.0))

    b_mlp = np.asarray(b_mlp, np.float32)
    has_bmlp = bool(np.any(b_mlp != 0.0))
    bmlp4 = np.tile(b_mlp[None, None, :], (128, 4, 1)).reshape(128, 256)

    bc_vec = (np.asarray(b_emb, np.float32) @ np.asarray(W_last, np.float32).T
              + np.asarray(b_last, np.float32))                  # [16]
    bc_t = np.tile(bc_vec[None, :], (128, 1)).astype(np.float32)

    # x pretransposed on host (pure layout): xt[p, h, n] = x[n, 128h + p]
    x_pad = np.zeros((NP_PAD, FEAT), np.float32)
    x_pad[:N_NODES] = np.asarray(x, np.float32)
    xt = np.ascontiguousarray(
        x_pad.T.reshape(2, 128, NP_PAD).transpose(1, 0, 2)
    ).astype(bf).reshape(128, 2 * NP_PAD)

    W_mlpT = np.asarray(W_mlp, np.float32).T                     # [256, 64]
    wmlp_pk = np.ascontiguousarray(
        W_mlpT.reshape(2, 128, HID).transpose(1, 0, 2)
    ).astype(bf).reshape(128, 2 * HID)

    plan = dict(E_pad=E_pad, n_tiles=n_tiles, n_ech=n_ech, pairs=pairs,
                pair_of=pair_of, last_k=last_k, flush_at=flush_at,
                n_pairs=n_pairs, has_bnhh=has_bnhh, has_bmlp=has_bmlp)
    shared = dict(xt=xt, wmlp=wmlp_pk, wih=wih_p, whh8=whh8_p, ba=ba_p,
                  brz=brz, bnih=bnih, bnhh=bnhh, bmlp=bmlp4, bc=bc_t)
    percore = dict(gidx=gidx, oneh=oneh)
    return plan, shared, percore


def _build(plan):
    from contextlib import ExitStack
    import concourse.bass as bass  # noqa: F401
    import concourse.tile as tile
    from concourse import bacc, mybir

    f32 = mybir.dt.float32
    bf16 = mybir.dt.bfloat16
    f8 = mybir.dt.float8e4
    i16 = mybir.dt.int16
    AF = mybir.ActivationFunctionType
    OP = mybir.AluOpType
    DR = mybir.MatmulPerfMode.DoubleRow
    P = 128

    E_pad, n_tiles = plan["E_pad"], plan["n_tiles"]
    pairs, pair_of = plan["pairs"], plan["pair_of"]
    flush_at = plan["flush_at"]
    has_bnhh, has_bmlp = plan["has_bnhh"], plan["has_bmlp"]
    chunk_pairs = {}
    for (k, j) in pairs:
        chunk_pairs.setdefault(k, []).append(j)

    nc = bacc.Bacc("TRN2", target_bir_lowering=False, debug=False)

    xt_d = nc.dram_tensor("xt", [P, 2 * NP_PAD], bf16, kind="ExternalInput")
    wmlp_d = nc.dram_tensor("wmlp", [P, 2 * HID], bf16, kind="ExternalInput")
    wih_d = nc.dram_tensor("wih", [HID, G3], bf16, kind="ExternalInput")
    whh8_d = nc.dram_tensor("whh8", [P, 4 * G3], f8, kind="ExternalInput")
    ba_d = nc.dram_tensor("ba", [P, 4 * 136], bf16, kind="ExternalInput")
    brz_d = nc.dram_tensor("brz", [P, 8], f32, kind="ExternalInput")
    bnih_d = nc.dram_tensor("bnih", [P, 4], f32, kind="ExternalInput")
    bnhh_d = nc.dram_tensor("bnhh", [P, 4], f32, kind="ExternalInput")
    bmlp_d = nc.dram_tensor("bmlp", [P, 256], f32, kind="ExternalInput")
    bc_d = nc.dram_tensor("bc", [P, OUT_DIM], f32, kind="ExternalInput")
    gidx_d = nc.dram_tensor("gidx", [n_tiles, P, (WALK * E_TILE) // 16], i16,
                            kind="ExternalInput")
    oneh_d = nc.dram_tensor("oneh", [max(plan["n_pairs"], 1), P, P], bf16,
                            kind="ExternalInput")
    out_d = nc.dram_tensor("out", [NODE_CHUNKS * P, OUT_DIM], f32,
                           kind="ExternalOutput")
    etab_d = nc.dram_tensor("etab", [NP_PAD, P], bf16, kind="Internal")

    from concourse.masks import make_identity

    with tile.TileContext(nc) as tc, ExitStack() as ctx:
        wpool = ctx.enter_context(tc.tile_pool(name="w", bufs=1))
        wih_sb = wpool.tile([HID, G3], bf16, tag="wih")
        nc.sync.dma_start(wih_sb[:], wih_d[:])
        whh8_sb = wpool.tile([P, 4 * G3], f8, tag="whh8")
        nc.sync.dma_start(whh8_sb[:], whh8_d[:])
        ba_sb = wpool.tile([P, 4 * 136], bf16, tag="ba")
        nc.sync.dma_start(ba_sb[:], ba_d[:])
        brz_sb = wpool.tile([P, 8], f32, tag="brz")
        nc.sync.dma_start(brz_sb[:], brz_d[:])
        bnih_sb = wpool.tile([P, 4], f32, tag="bnih")
        nc.sync.dma_start(bnih_sb[:], bnih_d[:])
        bnhh_sb = wpool.tile([P, 4], f32, tag="bnhh")
        nc.sync.dma_start(bnhh_sb[:], bnhh_d[:])
        bc_sb = wpool.tile([P, OUT_DIM], f32, tag="bc")
        nc.sync.dma_start(bc_sb[:], bc_d[:])
        wm_sb = wpool.tile([P, 2 * HID], bf16, tag="wm")
        nc.sync.dma_start(wm_sb[:], wmlp_d[:])
        if has_bmlp:
            bmlp_sb = wpool.tile([P, 256], f32, tag="bmlp")
            nc.sync.dma_start(bmlp_sb[:], bmlp_d[:])
        ident16 = wpool.tile([P, P], bf16, tag="ident16")
        make_identity(nc, ident16[:])
        # per-node-chunk [num(128) | den(8)] accumulators, f32 in SBUF
        ft_sb = wpool.tile([P, NODE_CHUNKS * 136], f32, tag="ft")
        nc.vector.memset(ft_sb[:], 0)

        whh8_v = whh8_sb[:].rearrange("p (k g) -> p k g", k=4)
        ba_v = ba_sb[:].rearrange("p (k b) -> p k b", k=4)
        xt_v = xt_d[:].rearrange("p (h n) -> p h n", h=2)
        wm_v = wm_sb[:].rearrange("p (h d) -> p h d", h=2)

        # ---------------- phase 1: embedding table ----------------
        with tc.tile_pool(name="e_sb", bufs=3) as epool, \
             tc.tile_pool(name="e_ps", bufs=2, space="PSUM") as epsum:
            n_grp = NP_PAD // 512  # 40 groups of 4 node-chunks
            for g in range(n_grp):
                ep = epsum.tile([P, 4, HID], f32, tag="ep", space="PSUM")
                for k in range(4):
                    cs = (g * 4 + k) * P
                    xsb = epool.tile([P, 2, P], bf16, tag="xsb")
                    nc.sync.dma_start(xsb[:], xt_v[:, :, cs:cs + P])
                    nc.tensor.matmul(ep[:, k, :], xsb[:, 0, :], wm_v[:, 0, :],
                                     start=True, stop=False)
                    nc.tensor.matmul(ep[:, k, :], xsb[:, 1, :], wm_v[:, 1, :],
                                     start=False, stop=True)
                esb = epool.tile([P, 4, HID], bf16, tag="esb")
                if has_bmlp:
                    nc.vector.tensor_tensor(
                        esb[:].rearrange("p k d -> p (k d)"),
                        ep[:].rearrange("p k d -> p (k d)"),
                        bmlp_sb[:], OP.add)
                else:
                    nc.vector.tensor_copy(
                        esb[:].rearrange("p k d -> p (k d)"),
                        ep[:].rearrange("p k d -> p (k d)"))
                dst = etab_d[g * 512:(g + 1) * 512, 0:HID]
                nc.sync.dma_start(
                    dst.rearrange("(k p) d -> p k d", p=P), esb[:])

        # -------- phase 2+3 fused: GRU + attention + scatter --------
        NIDX = WALK * E_TILE

        def wih_s(m):
            return wih_sb[:, m * P:(m + 1) * P]

        with tc.tile_pool(name="g_idx", bufs=2) as ipool, \
             tc.tile_pool(name="g_gat", bufs=2) as gpool, \
             tc.tile_pool(name="g_rz", bufs=6) as rzpool, \
             tc.tile_pool(name="g_n", bufs=4) as npool, \
             tc.tile_pool(name="g_hb", bufs=4) as hbpool, \
             tc.tile_pool(name="g_hq", bufs=4) as hqpool, \
             tc.tile_pool(name="g_tmp", bufs=5) as tpool, \
             tc.tile_pool(name="p3_pa", bufs=2) as p3pool, \
             tc.tile_pool(name="p3_oh", bufs=4) as ohpool, \
             tc.tile_pool(name="p3_sm", bufs=10) as spool, \
             tc.tile_pool(name="g_ps", bufs=4, space="PSUM") as gpsum, \
             tc.tile_pool(name="pa_ps", bufs=2, space="PSUM") as papsum, \
             tc.tile_pool(name="oh_ps", bufs=2, space="PSUM") as ohpsum:

            for t in range(n_tiles):
                idxt = ipool.tile([P, NIDX // 16], i16, tag="idx")
                nc.sync.dma_start(idxt[:], gidx_d[t])
                gat = gpool.tile([P, 1, NIDX], bf16, tag="gat")
                nc.gpsimd.dma_gather(gat[:], etab_d[:], idxt[:], NIDX, NIDX,
                                     P, transpose=True, single_packet=False)

                def x_s(s):
                    return gat[0:HID, 0, s * E_TILE:(s + 1) * E_TILE]

                # ---- step 0 (h = 0)
                zp = [rzpool.tile([P, 2, E_TILE], bf16, tag=f"z{a}", name=f"zp{a}")
                      for a in range(2)]
                np_ = [npool.tile([P, 2, E_TILE], bf16, tag=f"n{a}", name=f"np{a}")
                       for a in range(2)]
                r0 = [None] * 4
                if has_bnhh:
                    for c in range(4):
                        ps = gpsum.tile([P, E_TILE], f32, tag="g", space="PSUM")
                        nc.tensor.matmul(ps[:], wih_s(c), x_s(0),
                                         start=True, stop=True)
                        rt = tpool.tile([P, 2, E_TILE], bf16, tag="rt")
                        nc.scalar.activation(rt[:, 0, :], ps[:], AF.Sigmoid,
                                             bias=brz_sb[:, c:c + 1])
                        r0[c] = rt
                for c in range(4):
                    ps = gpsum.tile([P, E_TILE], f32, tag="g", space="PSUM")
                    nc.tensor.matmul(ps[:], wih_s(4 + c), x_s(0),
                                     start=True, stop=True)
                    nc.scalar.activation(zp[c // 2][:, c % 2, :], ps[:],
                                         AF.Sigmoid, bias=brz_sb[:, 4 + c:5 + c])
                for c in range(4):
                    ps = gpsum.tile([P, E_TILE], f32, tag="g", space="PSUM")
                    if has_bnhh:
                        nc.tensor.matmul(ps[:], wih_s(8 + c), x_s(0),
                                         start=True, stop=False)
                        rb = tpool.tile([P, E_TILE], bf16, tag="rb")
                        nc.vector.tensor_scalar(rb[:], r0[c][:, 0, :],
                                                bnhh_sb[:, c:c + 1], None,
                                                OP.mult)
                        nc.tensor.matmul(ps[:], ident16[:], rb[:],
                                         start=False, stop=True,
                                         skip_group_check=True)
                    else:
                        nc.tensor.matmul(ps[:], wih_s(8 + c), x_s(0),
                                         start=True, stop=True)
                    nc.scalar.activation(np_[c // 2][:, c % 2, :], ps[:],
                                         AF.Tanh, bias=bnih_sb[:, c:c + 1])
                hb = [hbpool.tile([P, 2, E_TILE], bf16, tag=f"hb{a}", name=f"hb{a}")
                      for a in range(2)]
                hq = [hqpool.tile([P, 2, E_TILE], f8, tag=f"hq{a}", name=f"hq{a}")
                      for a in range(2)]
                for a in range(2):
                    zn = tpool.tile([P, 2, E_TILE], bf16, tag="zn")
                    nc.vector.tensor_tensor(
                        zn[:].rearrange("p i e -> p (i e)"),
                        zp[a][:].rearrange("p i e -> p (i e)"),
                        np_[a][:].rearrange("p i e -> p (i e)"), OP.mult)
                    nc.vector.tensor_tensor(
                        hb[a][:].rearrange("p i e -> p (i e)"),
                        np_[a][:].rearrange("p i e -> p (i e)"),
                        zn[:].rearrange("p i e -> p (i e)"), OP.subtract)
                    nc.vector.tensor_copy(
                        hq[a][:].rearrange("p i e -> p (i e)"),
                        hb[a][:].rearrange("p i e -> p (i e)"))

                # ---- steps 1..3
                for s in range(1, WALK):
                    final = (s == WALK - 1)
                    rp = [rzpool.tile([P, 2, E_TILE], bf16, tag=f"r{a}", name=f"rp{a}")
                          for a in range(2)]
                    zp = [rzpool.tile([P, 2, E_TILE], bf16, tag=f"z{a}", name=f"zp{a}")
                          for a in range(2)]
                    np_ = [npool.tile([P, 2, E_TILE], bf16, tag=f"n{a}", name=f"np{a}")
                           for a in range(2)]
                    for m in range(8):
                        ps = gpsum.tile([P, E_TILE], f32, tag="g", space="PSUM")
                        nc.tensor.matmul(ps[:], wih_s(m), x_s(s),
                                         start=True, stop=False)
                        nc.tensor.matmul(ps[:], whh8_v[:, 0:2, m * P:(m + 1) * P],
                                         hq[0][:], start=False, stop=False,
                                         perf_mode=DR)
                        nc.tensor.matmul(ps[:], whh8_v[:, 2:4, m * P:(m + 1) * P],
                                         hq[1][:], start=False, stop=True,
                                         perf_mode=DR)
                        dstp = rp if m < 4 else zp
                        c = m % 4
                        nc.scalar.activation(dstp[c // 2][:, c % 2, :], ps[:],
                                             AF.Sigmoid, bias=brz_sb[:, m:m + 1])
                    for c in range(4):
                        m = 8 + c
                        php = gpsum.tile([P, E_TILE], f32, tag="g", space="PSUM")
                        nc.tensor.matmul(php[:], whh8_v[:, 0:2, m * P:(m + 1) * P],
                                         hq[0][:], start=True, stop=False,
                                         perf_mode=DR)
                        nc.tensor.matmul(php[:], whh8_v[:, 2:4, m * P:(m + 1) * P],
                                         hq[1][:], start=False, stop=True,
                                         perf_mode=DR)
                        pxp = gpsum.tile([P, E_TILE], f32, tag="g", space="PSUM")
                        nc.tensor.matmul(pxp[:], wih_s(m), x_s(s),
                                         start=True, stop=False)
                        rhn = tpool.tile([P, E_TILE], bf16, tag="rhn")
                        if has_bnhh:
                            phb = tpool.tile([P, E_TILE], f32, tag="phb")
                            nc.vector.tensor_scalar(phb[:], php[:],
                                                    bnhh_sb[:, c:c + 1], None,
                                                    OP.add)
                            nc.vector.tensor_tensor(rhn[:], rp[c // 2][:, c % 2, :],
                                                    phb[:], OP.mult)
                        else:
                            nc.vector.tensor_tensor(rhn[:], rp[c // 2][:, c % 2, :],
                                                    php[:], OP.mult)
                        nc.tensor.matmul(pxp[:], ident16[:], rhn[:],
                                         start=False, stop=True,
                                         skip_group_check=True)
                        nc.scalar.activation(np_[c // 2][:, c % 2, :], pxp[:],
                                             AF.Tanh, bias=bnih_sb[:, c:c + 1])
                    nhb = [hbpool.tile([P, 2, E_TILE], bf16, tag=f"hb{a}", name=f"hb{a}")
                           for a in range(2)]
                    nhq = None
                    if not final:
                        nhq = [hqpool.tile([P, 2, E_TILE], f8, tag=f"hq{a}", name=f"hq{a}")
                               for a in range(2)]
                    for a in range(2):
                        d = tpool.tile([P, 2, E_TILE], bf16, tag="d")
                        nc.vector.tensor_tensor(
                            d[:].rearrange("p i e -> p (i e)"),
                            hb[a][:].rearrange("p i e -> p (i e)"),
                            np_[a][:].rearrange("p i e -> p (i e)"), OP.subtract)
                        zd = tpool.tile([P, 2, E_TILE], bf16, tag="zd")
                        nc.vector.tensor_tensor(
                            zd[:].rearrange("p i e -> p (i e)"),
                            zp[a][:].rearrange("p i e -> p (i e)"),
                            d[:].rearrange("p i e -> p (i e)"), OP.mult)
                        nc.vector.tensor_tensor(
                            nhb[a][:].rearrange("p i e -> p (i e)"),
                            np_[a][:].rearrange("p i e -> p (i e)"),
                            zd[:].rearrange("p i e -> p (i e)"), OP.add)
                        if not final:
                            nc.vector.tensor_copy(
                                nhq[a][:].rearrange("p i e -> p (i e)"),
                                nhb[a][:].rearrange("p i e -> p (i e)"))
                    hb = nhb
                    hq = nhq

                # ---- phase 3 for this tile: attention + one-hot scatter
                pasb = p3pool.tile([P, 4, 136], f32, tag="pasb")
                for kl in range(4):
                    pa = papsum.tile([P, 136], f32, tag="pa", space="PSUM")
                    for cc in range(4):
                        nc.tensor.matmul(
                            pa[:],
                            hb[cc // 2][:, cc % 2, kl * P:(kl + 1) * P],
                            ba_v[:, cc, :],
                            start=(cc == 0), stop=(cc == 3))
                    nc.scalar.copy(pasb[:, kl, :], pa[:])
                aslc = pasb[:, :, 128:136]                     # [128, 4, 8]
                asb = spool.tile([P, 4, 8], f32, tag="asb")
                nc.vector.tensor_scalar(asb[:], aslc, 0.01, None, OP.mult)
                amx = spool.tile([P, 4, 8], f32, tag="amx")
                nc.vector.tensor_tensor(amx[:], aslc, asb[:], OP.max)
                th = spool.tile([P, 4, 8], f32, tag="th")
                nc.scalar.activation(th[:], amx[:], AF.Tanh, scale=0.5)
                nm = spool.tile([P, 4, 8], f32, tag="nm")
                nc.vector.tensor_scalar(nm[:], th[:], 1.0, None, OP.add)
                dn = spool.tile([P, 4, 8], f32, tag="dn")
                nc.vector.tensor_scalar(dn[:], th[:], -1.0, 1.0,
                                        OP.mult, OP.add)
                rdn = spool.tile([P, 4, 8], f32, tag="rdn")
                nc.vector.reciprocal_approx_fast(
                    out=rdn[:].rearrange("p k h -> p (k h)"),
                    in_=dn[:].rearrange("p k h -> p (k h)"))
                ea = spool.tile([P, 4, 8], f32, tag="ea")
                nc.vector.tensor_tensor(ea[:], nm[:], rdn[:], OP.mult)
                pay = p3pool.tile([P, 4, 136], bf16, tag="pay")
                nc.vector.tensor_tensor(
                    pay[:, :, 0:128].rearrange("p k (h i) -> p k h i", h=NH),
                    pasb[:, :, 0:128].rearrange("p k (h i) -> p k h i", h=NH),
                    ea[:, :, :, None].to_broadcast([P, 4, NH, OUT_DIM]),
                    OP.mult)
                nc.vector.tensor_copy(pay[:, :, 128:136], ea[:])

                # one-hot scatter: per-tile psum partials -> SBUF ft
                tj = {}
                for kl in range(4):
                    k = t * 4 + kl
                    for j in chunk_pairs.get(k, []):
                        tj.setdefault(j, []).append(kl)
                for j, kls in tj.items():
                    acc = ohpsum.tile([P, 136], f32, tag="acc", space="PSUM")
                    for i, kl in enumerate(kls):
                        pid = pair_of[(t * 4 + kl, j)]
                        oh = ohpool.tile([P, P], bf16, tag="oh")
                        nc.sync.dma_start(oh[:], oneh_d[pid])
                        nc.tensor.matmul(acc[:], oh[:], pay[:, kl, :],
                                         start=(i == 0),
                                         stop=(i == len(kls) - 1),
                                         skip_group_check=True)
                    fts = ft_sb[:, j * 136:(j + 1) * 136]
                    nc.vector.tensor_tensor(fts, fts, acc[:], OP.add)

                # flush completed node chunks
                for j in flush_at.get(t, []):
                    dj = ft_sb[:, j * 136 + 128:j * 136 + 136]
                    sc = spool.tile([P, 8], f32, tag="sc")
                    nc.vector.tensor_scalar(sc[:], dj, 1e-30, None, OP.max)
                    rc = spool.tile([P, 8], f32, tag="rc")
                    nc.vector.reciprocal_approx_fast(out=rc[:], in_=sc[:])
                    wq = spool.tile([P, P], f32, tag="wq")
                    nc.vector.tensor_tensor(
                        wq[:].rearrange("p (h i) -> p h i", h=NH),
                        ft_sb[:, j * 136:j * 136 + 128]
                            .rearrange("p (h i) -> p h i", h=NH),
                        rc[:, :, None].to_broadcast([P, NH, OUT_DIM]),
                        OP.mult)
                    o16 = spool.tile([P, OUT_DIM], f32, tag="o16")
                    nc.vector.reduce_sum(
                        o16[:], wq[:].rearrange("p (h i) -> p i h", h=NH),
                        axis=mybir.AxisListType.X)
                    ob = spool.tile([P, OUT_DIM], f32, tag="ob")
                    nc.vector.tensor_tensor(ob[:], o16[:], bc_sb[:], OP.add)
                    nc.sync.dma_start(out_d[j * P:(j + 1) * P, :], ob[:])

    nc.compile()
    return nc


def kernel(**inputs):
    import os
    from concourse.bass_utils import run_bass_kernel_spmd

    num_nodes = int(inputs.pop("num_nodes", N_NODES))
    assert num_nodes == N_NODES
    plan, shared, percore = _host_prep(**inputs)
    nc = _build(plan)

    in_maps = []
    for c in range(N_CORES):
        m = dict(shared)
        m["gidx"] = np.ascontiguousarray(percore["gidx"][c])
        m["oneh"] = np.ascontiguousarray(percore["oneh"][c])
        in_maps.append(m)

    trace = bool(os.environ.get("KERNEL_TRACE"))
    res = run_bass_kernel_spmd(nc, in_maps, core_ids=list(range(N_CORES)),
                               trace=trace)
    global LAST_EXEC_NS, LAST_RESULTS
    LAST_EXEC_NS = getattr(res, "exec_time_ns", None)
    LAST_RESULTS = res

    full = np.empty((N_NODES, OUT_DIM), np.float32)
    for c in range(N_CORES):
        full[c * NPC:(c + 1) * NPC] = res.results[c]["out"][:NPC]
    return full
